# revision 3
# baseline (speedup 1.0000x reference)
"""Trainium2 Bass kernel for the 2-layer LSTM greedy decoder (nn_Decoder).

Strategy: data-parallel over batch (4096 -> 512 per core x 8 cores).
All recurrent state is kept feature-major in SBUF ([H partitions, batch
free]) so gate matmuls need no transposes; LSTM weights are pre-packed on
the host into the exact SBUF tile layout and streamed from HBM every step
(weights don't fit in SBUF in fp32). Matmuls run in float32r (fp32
storage, FP22 multiply) at full PE rate. Argmax feedback is folded into
the layer-0 gate accumulation as a rank-2 matmul update (ones/m rows);
biases ride the scalar-engine activation's per-partition bias operand.
"""

import os
import sys

sys.path.insert(0, "/opt/trn_rl_repo")

import numpy as np

import concourse.bass as bass
import concourse.bacc as bacc
import concourse.mybir as mybir
import concourse.tile as tile
from concourse.bass_utils import run_bass_kernel_spmd

F32 = mybir.dt.float32
F32R = mybir.dt.float32r
AF = mybir.ActivationFunctionType
ALU = mybir.AluOpType

H = 1024
B = 4096
C = 2
NCORES = 8
BS = B // NCORES          # 512 batch per core
KT = H // 128             # 8 k-tiles
HB = H // 128             # 8 hidden blocks
NQ = 4                    # i, f, g, o


def _round_f32r(x: np.ndarray) -> np.ndarray:
    """Round fp32 to the PE's FP22 (13-bit mantissa) operand precision."""
    u = np.ascontiguousarray(x, dtype=np.float32).view(np.uint32)
    u = (u + np.uint32(0x200)) & np.uint32(0xFFFFFC00)
    return u.view(np.float32)


def build_kernel(T: int, unroll_all: bool = False):
    nc = bacc.Bacc(None, target_bir_lowering=False)

    hs = nc.dram_tensor("hs", [2, 128, KT, BS], F32R, kind="ExternalInput")
    cs = nc.dram_tensor("cs", [2, 128, KT, BS], F32, kind="ExternalInput")
    w0p = nc.dram_tensor("w0p", [HB, NQ, 128, KT, 128], F32R, kind="ExternalInput")
    w1p = nc.dram_tensor("w1p", [HB, NQ, 128, 2 * KT, 128], F32R, kind="ExternalInput")
    xf = nc.dram_tensor("xf", [2, HB * NQ, 128], F32R, kind="ExternalInput")
    b0 = nc.dram_tensor("b0", [128, HB * NQ], F32, kind="ExternalInput")
    b1 = nc.dram_tensor("b1", [128, HB * NQ], F32, kind="ExternalInput")
    fcw = nc.dram_tensor("fcw", [128, KT, 3], F32R, kind="ExternalInput")
    fcb = nc.dram_tensor("fcb", [1, 3], F32R, kind="ExternalInput")
    ident = nc.dram_tensor("ident", [128, 128], F32, kind="ExternalInput")
    mu0 = nc.dram_tensor("mu0", [2, BS], F32R, kind="ExternalInput")
    lout = nc.dram_tensor("lout", [4, 128, T, 2], F32, kind="ExternalOutput")

    with tile.TileContext(nc) as tc:
        with (
            tc.tile_pool(name="st", bufs=1) as st,
            tc.tile_pool(name="wst", bufs=3) as wst,
            tc.tile_pool(name="tmp", bufs=2) as tmp,
            tc.tile_pool(name="gps", bufs=6, space="PSUM") as gps,
            tc.tile_pool(name="lps", bufs=1, space="PSUM") as lpsp,
            tc.tile_pool(name="tps", bufs=1, space="PSUM") as tpsp,
        ):
            # Persistent state (ping-pong h buffers; c updated in place)
            h0a = st.tile([128, KT, BS], F32R, tag="h0a")
            h0b = st.tile([128, KT, BS], F32R, tag="h0b")
            h1a = st.tile([128, KT, BS], F32R, tag="h1a")
            h1b = st.tile([128, KT, BS], F32R, tag="h1b")
            c0 = st.tile([128, KT, BS], F32, tag="c0")
            c1 = st.tile([128, KT, BS], F32, tag="c1")
            xf_sb = st.tile([2, HB * NQ * 128], F32R, tag="xf")
            b0_sb = st.tile([128, HB * NQ], F32, tag="b0")
            b1_sb = st.tile([128, HB * NQ], F32, tag="b1")
            fcw_sb = st.tile([128, KT, 3], F32R, tag="fcw")
            fcb_sb = st.tile([1, 3], F32R, tag="fcb")
            id_sb = st.tile([128, 128], F32, tag="ident")
            mu_sb = st.tile([2, BS], F32R, tag="mu")   # row0 = m, row1 = ones
            ones_sb = st.tile([1, BS], F32R, tag="ones")
            lt_sb = st.tile([3, BS], F32, tag="lt")    # rows d, l0, l1
            lacc = st.tile([128, 4 * T * 2], F32, tag="lacc")

            nc.sync.dma_start(h0a[:], hs[0])
            nc.sync.dma_start(h1a[:], hs[1])
            nc.sync.dma_start(c0[:], cs[0])
            nc.sync.dma_start(c1[:], cs[1])
            nc.sync.dma_start(xf_sb[:], xf.rearrange("r t f -> r (t f)"))
            nc.sync.dma_start(b0_sb[:], b0[:])
            nc.sync.dma_start(b1_sb[:], b1[:])
            nc.sync.dma_start(fcw_sb[:], fcw[:])
            nc.sync.dma_start(fcb_sb[:], fcb[:])
            nc.sync.dma_start(id_sb[:], ident[:])
            # row1 stays 1.0 forever; row0 (m) is overwritten by is_lt each
            # step before any matmul reads it (step 0 skips the m-matmul).
            nc.sync.dma_start(mu_sb[:], mu0[:])
            nc.sync.dma_start(ones_sb[:], mu0[0:1])

            def phase_gates(layer, j, h_in, h_aux, with_m):
                """Gate matmuls + activations for hidden block j of one layer.

                layer 0: contraction = W_hh0 @ h_in (+ x feedback via m-matmul)
                layer 1: contraction = W_hh1 @ h_in then W_ih1 @ h_aux
                """
                nkt = KT if layer == 0 else 2 * KT
                wsrc = w0p if layer == 0 else w1p
                bias = b0_sb if layer == 0 else b1_sb
                gact = tmp.tile([128, NQ, BS], F32, tag="gact")
                for q in range(NQ):
                    w = wst.tile([128, nkt, 128], F32R, tag="w")
                    nc.sync.dma_start(w[:], wsrc[j, q])
                    ps = gps.tile([128, BS], F32, tag="g")
                    for kt in range(nkt):
                        rhs = h_in if kt < KT else h_aux
                        last = (kt == nkt - 1) and not (layer == 0 and with_m)
                        nc.tensor.matmul(
                            ps[:],
                            w[:, kt, :],
                            rhs[:, kt % KT, :],
                            start=(kt == 0),
                            stop=last,
                        )
                    if layer == 0 and with_m:
                        idx = j * NQ + q
                        nc.tensor.matmul(
                            ps[:],
                            xf_sb[:, idx * 128:(idx + 1) * 128],
                            mu_sb[:],
                            start=False,
                            stop=True,
                        )
                    fn = AF.Tanh if q == 2 else AF.Sigmoid
                    idx = j * NQ + q
                    nc.scalar.activation(
                        gact[:, q, :], ps[:], fn, bias=bias[:, idx:idx + 1]
                    )
                return gact

            def phase_cell(j, gact, c_st, h_out):
                t1 = tmp.tile([128, BS], F32, tag="t1")
                t2 = tmp.tile([128, BS], F32, tag="t2")
                ct = tmp.tile([128, BS], F32, tag="ct")
                nc.vector.tensor_mul(t1[:], gact[:, 1, :], c_st[:, j, :])
                nc.vector.tensor_mul(t2[:], gact[:, 0, :], gact[:, 2, :])
                nc.vector.tensor_add(c_st[:, j, :], t1[:], t2[:])
                nc.scalar.activation(ct[:], c_st[:, j, :], AF.Tanh)
                nc.vector.tensor_mul(h_out[:, j, :], gact[:, 3, :], ct[:])

            def emit_step(t_off, h0_in, h0_out, h1_in, h1_out, first):
                # one-block skew: block j's cell tail is emitted after block
                # j+1's gate phase, keeping tanh(c)'s DVE-wait off the ACT
                # engine's head-of-line and freeing PSUM banks promptly.
                prev = None
                for j in range(HB):
                    g = phase_gates(0, j, h0_in, None, not first)
                    if prev is not None:
                        phase_cell(prev[0], prev[1], c0, h0_out)
                    prev = (j, g)
                phase_cell(prev[0], prev[1], c0, h0_out)
                prev = None
                for j in range(HB):
                    g = phase_gates(1, j, h1_in, h0_out, False)
                    if prev is not None:
                        phase_cell(prev[0], prev[1], c1, h1_out)
                    prev = (j, g)
                phase_cell(prev[0], prev[1], c1, h1_out)
                # logits: [d; l0; l1] = [fcW0-fcW1; fcW0; fcW1] @ h1_out + bias row
                lps = lpsp.tile([3, BS], F32, tag="l")
                for kt in range(KT):
                    nc.tensor.matmul(
                        lps[:], fcw_sb[:, kt, :], h1_out[:, kt, :],
                        start=(kt == 0), stop=False,
                    )
                nc.tensor.matmul(
                    lps[:], fcb_sb[:], ones_sb[:], start=False, stop=True
                )
                # m = 1.0 if l1 > l0 else 0.0  (d = l0 - l1 < 0)
                nc.vector.tensor_scalar(
                    mu_sb[0:1, :], lps[0:1, :], 0.0, None, ALU.is_lt
                )
                nc.vector.tensor_copy(lt_sb[:], lps[:])
                tp = tpsp.tile([128, 4 * 3], F32, tag="tp")
                for bt in range(4):
                    nc.tensor.transpose(
                        tp[:, bt * 3:(bt + 1) * 3],
                        lt_sb[:, bt * 128:(bt + 1) * 128],
                        id_sb[0:3, 0:3],
                    )
                for bt in range(4):
                    nc.vector.tensor_copy(
                        lacc[:, bass.ds(bt * T * 2 + t_off * 2, 2)],
                        tp[:, bt * 3 + 1:bt * 3 + 3],
                    )

            # step 0 (x = zeros: no m-matmul)
            emit_step(0, h0a, h0b, h1a, h1b, first=True)
            if T < 8 or T % 2 != 0:
                unroll_all = True
            if unroll_all:
                for t in range(1, T):
                    if t % 2 == 1:
                        emit_step(t, h0b, h0a, h1b, h1a, first=False)
                    else:
                        emit_step(t, h0a, h0b, h1a, h1b, first=False)
            else:
                # steps 1..T-2 in ping-pong pairs
                with tc.For_i(1, T - 1, 2) as i:
                    emit_step(i, h0b, h0a, h1b, h1a, first=False)
                    emit_step(i + 1, h0a, h0b, h1a, h1b, first=False)
                # step T-1
                emit_step(T - 1, h0b, h0a, h1b, h1a, first=False)

            for bt in range(4):
                nc.sync.dma_start(
                    lout[bt].rearrange("p t c -> p (t c)"),
                    lacc[:, bt * T * 2:(bt + 1) * T * 2],
                )
    nc.compile()
    return nc


def pack_inputs(h, c, W_ih0, W_hh0, b_ih0, b_hh0, W_ih1, W_hh1, b_ih1, b_hh1,
                fc_W, fc_b, T):
    """Host-side packing into per-core input maps (all exact SBUF layouts)."""
    h = np.asarray(h, np.float32)
    c = np.asarray(c, np.float32)

    def gate_pack(W, nkt_half=False):
        # -> [HB, NQ, 128(p=k), KT(kt), 128(f=g)]:  W[q*1024+j*128+f, kt*128+p]
        Wr = np.asarray(W, np.float32).reshape(NQ, HB, 128, KT, 128)  # q j f kt p
        return np.ascontiguousarray(Wr.transpose(1, 0, 4, 3, 2))

    w0 = _round_f32r(gate_pack(W_hh0))                       # [8,4,128,8,128]
    w1h = gate_pack(W_hh1)
    w1i = gate_pack(W_ih1)
    w1 = _round_f32r(np.concatenate([w1h, w1i], axis=3))     # [8,4,128,16,128]

    # x-feedback lhsT rows: row0 = B-A (pairs with m), row1 = A (pairs with ones)
    Wi0 = np.asarray(W_ih0, np.float32).reshape(NQ, HB, 128, C)  # q j f c
    A = Wi0[..., 0].transpose(1, 0, 2).reshape(HB * NQ, 128)
    BA = (Wi0[..., 1] - Wi0[..., 0]).transpose(1, 0, 2).reshape(HB * NQ, 128)
    xf = _round_f32r(np.stack([BA, A], axis=0))              # [2, 32, 128]

    def bias_pack(bi, bh):
        s = (np.asarray(bi, np.float32) + np.asarray(bh, np.float32))
        return np.ascontiguousarray(
            s.reshape(NQ, HB, 128).transpose(2, 1, 0).reshape(128, HB * NQ))

    b0 = bias_pack(b_ih0, b_hh0)
    b1 = bias_pack(b_ih1, b_hh1)

    fc_W = np.asarray(fc_W, np.float32)
    fc_b = np.asarray(fc_b, np.float32)
    # columns [d, l0, l1]
    cols = np.stack([fc_W[0] - fc_W[1], fc_W[0], fc_W[1]], axis=1)  # [H, 3]
    fcw = _round_f32r(np.ascontiguousarray(
        cols.reshape(KT, 128, 3).transpose(1, 0, 2)))               # [128, 8, 3]
    fcb = _round_f32r(
        np.array([[fc_b[0] - fc_b[1], fc_b[0], fc_b[1]]], np.float32))
    ident = np.eye(128, dtype=np.float32)

    hT = h.transpose(0, 2, 1).reshape(2, KT, 128, B)   # [l, kt, p, b]
    cT = c.transpose(0, 2, 1).reshape(2, KT, 128, B)

    in_maps = []
    for i in range(NCORES):
        sl = slice(i * BS, (i + 1) * BS)
        in_maps.append({
            "hs": _round_f32r(np.ascontiguousarray(
                hT[:, :, :, sl].transpose(0, 2, 1, 3))),   # [2,128,KT,BS]
            "cs": np.ascontiguousarray(cT[:, :, :, sl].transpose(0, 2, 1, 3)),
            "w0p": w0, "w1p": w1, "xf": xf, "b0": b0, "b1": b1,
            "fcw": fcw, "fcb": fcb, "ident": ident,
            "mu0": np.ones((2, BS), np.float32),
        })
    return in_maps


_CACHE = {}


def _run(inputs, trace=False, tmpdir=None):
    T = int(inputs["pred_len"])
    if T not in _CACHE:
        _CACHE[T] = build_kernel(T)
    nc = _CACHE[T]
    in_maps = pack_inputs(
        inputs["h"], inputs["c"], inputs["W_ih0"], inputs["W_hh0"],
        inputs["b_ih0"], inputs["b_hh0"], inputs["W_ih1"], inputs["W_hh1"],
        inputs["b_ih1"], inputs["b_hh1"], inputs["fc_W"], inputs["fc_b"], T)
    res = run_bass_kernel_spmd(
        nc, in_maps, core_ids=list(range(NCORES)), trace=trace, tmpdir=tmpdir)
    out = np.empty((B, T, C), np.float32)
    for i in range(NCORES):
        lo = res.results[i]["lout"]                    # [4, 128, T, 2]
        out[i * BS:(i + 1) * BS] = lo.reshape(BS, T, C)
    return out, res


def kernel(**inputs) -> np.ndarray:
    out, _ = _run(inputs, trace=False)
    return out



# revision 4
# speedup vs baseline: 6.9379x; 6.9379x over previous
"""Trainium2 Bass kernel v4: bf16 + fused activations.

v3 measured ~320ns/MM vs 204ns for a pure MM stream; microbenches showed
the gap is per-psum-group ACT coupling (~625ns/group), not LDWEIGHTS. v4
cuts psum-group count 4x: the three sigmoid gates (i,f,o) of each hidden
block accumulate into one 3-bank psum group read by a single wide
sigmoid-ACT; the tanh gate gets its own bank. Biases move off the ACT
into the rank-2 ones-row matmuls (layer-0 folds bias into the x-feedback
A row for free; layer-1 sig gates get rank-1 bias matmuls; tanh gates
keep the ACT bias operand). Gate order is repacked (i,f,o,g) so each
block does sig,tanh,tanh activations: 2 table swaps per block.
"""

import os
import sys

sys.path.insert(0, "/opt/trn_rl_repo")

import ml_dtypes
import numpy as np

import concourse.bass as bass
import concourse.bacc as bacc
import concourse.mybir as mybir
import concourse.tile as tile
from concourse.bass_utils import run_bass_kernel_spmd

F32 = mybir.dt.float32
BF16 = mybir.dt.bfloat16
AF = mybir.ActivationFunctionType
ALU = mybir.AluOpType
BF = ml_dtypes.bfloat16

H = 1024
B = 4096
C = 2
NCORES = 8
BS = B // NCORES
KT = H // 128
HB = H // 128
NQ = 4          # gate order AFTER repack: i, f, o, g  (sig, sig, sig, tanh)


def build_kernel(T: int):
    nc = bacc.Bacc(None, target_bir_lowering=False)

    hs = nc.dram_tensor("hs", [2, 128, KT, BS], BF16, kind="ExternalInput")
    cs = nc.dram_tensor("cs", [2, 128, KT, BS], F32, kind="ExternalInput")
    w0p = nc.dram_tensor("w0p", [HB, NQ, 128, KT, 128], BF16,
                         kind="ExternalInput")
    w1p = nc.dram_tensor("w1p", [HB, NQ, 128, 2 * KT, 128], BF16,
                         kind="ExternalInput")
    xf = nc.dram_tensor("xf", [2, HB * NQ, 128], BF16, kind="ExternalInput")
    bs0 = nc.dram_tensor("bs0", [1, HB * NQ * 128], BF16,
                         kind="ExternalInput")
    bs1 = nc.dram_tensor("bs1", [1, HB * 3 * 128], BF16,
                         kind="ExternalInput")
    b1g = nc.dram_tensor("b1g", [128, HB], F32, kind="ExternalInput")
    fcw = nc.dram_tensor("fcw", [128, KT, 3], BF16, kind="ExternalInput")
    fcb = nc.dram_tensor("fcb", [1, 3], BF16, kind="ExternalInput")
    ident = nc.dram_tensor("ident", [128, 128], F32, kind="ExternalInput")
    mu0 = nc.dram_tensor("mu0", [2, BS], BF16, kind="ExternalInput")
    lout = nc.dram_tensor("lout", [4, 128, T, 2], F32, kind="ExternalOutput")

    with tile.TileContext(nc) as tc:
        with (
            tc.tile_pool(name="st", bufs=1) as st,
            tc.tile_pool(name="wst", bufs=4) as wst,
            tc.tile_pool(name="tmp", bufs=2) as tmp,
            tc.tile_pool(name="sigp", bufs=1, space="PSUM") as sigp,
            tc.tile_pool(name="tanp", bufs=3, space="PSUM") as tanp,
            tc.tile_pool(name="lps", bufs=1, space="PSUM") as lpsp,
            tc.tile_pool(name="tps", bufs=1, space="PSUM") as tpsp,
        ):
            h0a = st.tile([128, KT, BS], BF16, tag="h0a")
            h0b = st.tile([128, KT, BS], BF16, tag="h0b")
            h1a = st.tile([128, KT, BS], BF16, tag="h1a")
            h1b = st.tile([128, KT, BS], BF16, tag="h1b")
            c0 = st.tile([128, KT, BS], F32, tag="c0")
            c1 = st.tile([128, KT, BS], F32, tag="c1")
            xf_sb = st.tile([2, HB * NQ * 128], BF16, tag="xf")
            bs0_sb = st.tile([1, HB * NQ * 128], BF16, tag="bs0")
            bs1_sb = st.tile([1, HB * 3 * 128], BF16, tag="bs1")
            b1g_sb = st.tile([128, HB], F32, tag="b1g")
            fcw_sb = st.tile([128, KT, 3], BF16, tag="fcw")
            fcb_sb = st.tile([1, 3], BF16, tag="fcb")
            id_sb = st.tile([128, 128], F32, tag="ident")
            mu_sb = st.tile([2, BS], BF16, tag="mu")
            ones_sb = st.tile([1, BS], BF16, tag="ones")
            lt_sb = st.tile([3, BS], F32, tag="lt")
            lacc = st.tile([128, 4 * T * 2], F32, tag="lacc")

            nc.sync.dma_start(h0a[:], hs[0])
            nc.sync.dma_start(h1a[:], hs[1])
            nc.sync.dma_start(c0[:], cs[0])
            nc.sync.dma_start(c1[:], cs[1])
            nc.sync.dma_start(xf_sb[:], xf.rearrange("r t f -> r (t f)"))
            nc.sync.dma_start(bs0_sb[:], bs0[:])
            nc.sync.dma_start(bs1_sb[:], bs1[:])
            nc.sync.dma_start(b1g_sb[:], b1g[:])
            nc.sync.dma_start(fcw_sb[:], fcw[:])
            nc.sync.dma_start(fcb_sb[:], fcb[:])
            nc.sync.dma_start(id_sb[:], ident[:])
            nc.sync.dma_start(mu_sb[:], mu0[:])
            nc.sync.dma_start(ones_sb[:], mu0[0:1])

            def gates(layer, j, h_in, h_aux, first):
                """Emit one block's gate matmuls + fused activations."""
                nkt = KT if layer == 0 else 2 * KT
                wsrc = w0p if layer == 0 else w1p
                sps = sigp.tile([128, 3, BS], F32, tag="sig")
                tps_g = tanp.tile([128, BS], F32, tag="tan")
                gact = tmp.tile([128, NQ, BS], F32, tag="gact")

                def kt_mms(q, ps, stop_last):
                    w = wst.tile([128, nkt, 128], BF16, tag="w")
                    nc.sync.dma_start(w[:], wsrc[j, q])
                    for kt in range(nkt):
                        rhs = h_in if kt < KT else h_aux
                        nc.tensor.matmul(
                            ps, w[:, kt, :], rhs[:, kt % KT, :],
                            start=(kt == 0),
                            stop=stop_last and (kt == nkt - 1),
                        )

                # tanh gate first: gives the next block's sig-ACT drain cover
                l1_tanh_closes = (layer == 1)
                kt_mms(3, tps_g[:], l1_tanh_closes)
                for q in range(3):
                    kt_mms(q, sps[:, q, :], False)
                # closers
                if layer == 0:
                    src, rhs = (bs0_sb, ones_sb) if first else (xf_sb, mu_sb)
                    for q in range(3):
                        idx = j * NQ + q
                        nc.tensor.matmul(
                            sps[:, q, :],
                            src[:, idx * 128:(idx + 1) * 128],
                            rhs[:],
                            start=False, stop=True,
                        )
                    idx = j * NQ + 3
                    nc.tensor.matmul(
                        tps_g[:],
                        src[:, idx * 128:(idx + 1) * 128],
                        rhs[:],
                        start=False, stop=True,
                    )
                else:
                    for q in range(3):
                        idx = j * 3 + q
                        nc.tensor.matmul(
                            sps[:, q, :],
                            bs1_sb[:, idx * 128:(idx + 1) * 128],
                            ones_sb[:],
                            start=False, stop=True,
                        )
                nc.scalar.activation(gact[:, 0:3, :], sps[:], AF.Sigmoid)
                if layer == 1:
                    nc.scalar.activation(
                        gact[:, 3, :], tps_g[:], AF.Tanh,
                        bias=b1g_sb[:, j:j + 1])
                else:
                    nc.scalar.activation(gact[:, 3, :], tps_g[:], AF.Tanh)
                return gact

            def cell(j, gact, c_st, h_out):
                # gate slots: 0=i, 1=f, 2=o, 3=g
                t1 = tmp.tile([128, BS], F32, tag="t1")
                t2 = tmp.tile([128, BS], F32, tag="t2")
                ct = tmp.tile([128, BS], F32, tag="ct")
                nc.vector.tensor_mul(t1[:], gact[:, 1, :], c_st[:, j, :])
                nc.vector.tensor_mul(t2[:], gact[:, 0, :], gact[:, 3, :])
                nc.vector.tensor_add(c_st[:, j, :], t1[:], t2[:])
                nc.scalar.activation(ct[:], c_st[:, j, :], AF.Tanh)
                nc.vector.tensor_mul(h_out[:, j, :], gact[:, 2, :], ct[:])

            def emit_step(t_off, h0_in, h0_out, h1_in, h1_out, first):
                prev = None
                for j in range(HB):
                    g = gates(0, j, h0_in, None, first)
                    if prev is not None:
                        cell(prev[0], prev[1], c0, h0_out)
                    prev = (j, g)
                cell(prev[0], prev[1], c0, h0_out)
                prev = None
                for j in range(HB):
                    g = gates(1, j, h1_in, h0_out, False)
                    if prev is not None:
                        cell(prev[0], prev[1], c1, h1_out)
                    prev = (j, g)
                cell(prev[0], prev[1], c1, h1_out)
                lps = lpsp.tile([3, BS], F32, tag="l")
                for kt in range(KT):
                    nc.tensor.matmul(
                        lps[:], fcw_sb[:, kt, :], h1_out[:, kt, :],
                        start=(kt == 0), stop=False,
                    )
                nc.tensor.matmul(
                    lps[:], fcb_sb[:], ones_sb[:], start=False, stop=True
                )
                nc.vector.tensor_scalar(
                    mu_sb[0:1, :], lps[0:1, :], 0.0, None, ALU.is_lt
                )
                nc.vector.tensor_copy(lt_sb[:], lps[:])
                tp = tpsp.tile([128, 4 * 3], F32, tag="tp")
                for bt in range(4):
                    nc.tensor.transpose(
                        tp[:, bt * 3:(bt + 1) * 3],
                        lt_sb[:, bt * 128:(bt + 1) * 128],
                        id_sb[0:3, 0:3],
                    )
                for bt in range(4):
                    nc.vector.tensor_copy(
                        lacc[:, bass.ds(bt * T * 2 + t_off * 2, 2)],
                        tp[:, bt * 3 + 1:bt * 3 + 3],
                    )

            emit_step(0, h0a, h0b, h1a, h1b, first=True)
            if T < 8 or T % 2 != 0:
                for t in range(1, T):
                    if t % 2 == 1:
                        emit_step(t, h0b, h0a, h1b, h1a, first=False)
                    else:
                        emit_step(t, h0a, h0b, h1a, h1b, first=False)
            else:
                with tc.For_i(1, T - 1, 2) as i:
                    emit_step(i, h0b, h0a, h1b, h1a, first=False)
                    emit_step(i + 1, h0a, h0b, h1a, h1b, first=False)
                emit_step(T - 1, h0b, h0a, h1b, h1a, first=False)

            for bt in range(4):
                nc.sync.dma_start(
                    lout[bt].rearrange("p t c -> p (t c)"),
                    lacc[:, bt * T * 2:(bt + 1) * T * 2],
                )
    nc.compile()
    return nc


SEL = [0, 1, 3, 2]   # torch (i,f,g,o) -> packed (i,f,o,g)


def pack_inputs(h, c, W_ih0, W_hh0, b_ih0, b_hh0, W_ih1, W_hh1, b_ih1, b_hh1,
                fc_W, fc_b, T):
    h = np.asarray(h, np.float32)
    c = np.asarray(c, np.float32)

    def gate_pack(W):
        # -> [HB, NQ(packed), 128(p=k), KT, 128(f=gate-rows)]
        Wr = np.asarray(W, np.float32).reshape(NQ, HB, 128, KT, 128)[SEL]
        return np.ascontiguousarray(Wr.transpose(1, 0, 4, 3, 2))

    w0 = gate_pack(W_hh0).astype(BF)
    w1 = np.concatenate(
        [gate_pack(W_hh1), gate_pack(W_ih1)], axis=3).astype(BF)

    btot0 = (np.asarray(b_ih0, np.float32) + np.asarray(b_hh0, np.float32))
    btot1 = (np.asarray(b_ih1, np.float32) + np.asarray(b_hh1, np.float32))
    # packed-gate-order views [NQ, HB, 128]
    b0q = btot0.reshape(NQ, HB, 128)[SEL]
    b1q = btot1.reshape(NQ, HB, 128)[SEL]

    Wi0 = np.asarray(W_ih0, np.float32).reshape(NQ, HB, 128, C)[SEL]
    A = Wi0[..., 0] + b0q                          # bias folded into A row
    BA = Wi0[..., 1] - Wi0[..., 0]
    xfv = np.stack([
        BA.transpose(1, 0, 2).reshape(HB * NQ, 128),
        A.transpose(1, 0, 2).reshape(HB * NQ, 128),
    ], axis=0).astype(BF)                          # [2, 32, 128]

    bs0 = b0q.transpose(1, 0, 2).reshape(1, HB * NQ * 128).astype(BF)
    bs1 = b1q[0:3].transpose(1, 0, 2).reshape(1, HB * 3 * 128).astype(BF)
    b1gp = np.ascontiguousarray(b1q[3].T)          # [128, HB] f32

    fc_W = np.asarray(fc_W, np.float32)
    fc_b = np.asarray(fc_b, np.float32)
    cols = np.stack([fc_W[0] - fc_W[1], fc_W[0], fc_W[1]], axis=1)
    fcwp = np.ascontiguousarray(
        cols.reshape(KT, 128, 3).transpose(1, 0, 2)).astype(BF)
    fcbp = np.array(
        [[fc_b[0] - fc_b[1], fc_b[0], fc_b[1]]], np.float32).astype(BF)
    identp = np.eye(128, dtype=np.float32)

    hT = h.transpose(0, 2, 1).reshape(2, KT, 128, B)
    cT = c.transpose(0, 2, 1).reshape(2, KT, 128, B)

    in_maps = []
    for i in range(NCORES):
        sl = slice(i * BS, (i + 1) * BS)
        in_maps.append({
            "hs": np.ascontiguousarray(
                hT[:, :, :, sl].transpose(0, 2, 1, 3)).astype(BF),
            "cs": np.ascontiguousarray(cT[:, :, :, sl].transpose(0, 2, 1, 3)),
            "w0p": w0, "w1p": w1, "xf": xfv,
            "bs0": bs0, "bs1": bs1, "b1g": b1gp,
            "fcw": fcwp, "fcb": fcbp, "ident": identp,
            "mu0": np.ones((2, BS), BF),
        })
    return in_maps


_CACHE = {}


def kernel(**inputs) -> np.ndarray:
    T = int(inputs["pred_len"])
    if T not in _CACHE:
        _CACHE[T] = build_kernel(T)
    nc = _CACHE[T]
    in_maps = pack_inputs(
        inputs["h"], inputs["c"], inputs["W_ih0"], inputs["W_hh0"],
        inputs["b_ih0"], inputs["b_hh0"], inputs["W_ih1"], inputs["W_hh1"],
        inputs["b_ih1"], inputs["b_hh1"], inputs["fc_W"], inputs["fc_b"], T)
    res = run_bass_kernel_spmd(
        nc, in_maps, core_ids=list(range(NCORES)))
    out = np.empty((B, T, C), np.float32)
    for i in range(NCORES):
        lo = res.results[i]["lout"]
        out[i * BS:(i + 1) * BS] = lo.reshape(BS, T, C)
    return out
